# revision 1
# baseline (speedup 1.0000x reference)
"""Trainium2 Bass kernel for nn_Attention (B=4, S=2048, D=1024, H=16) on 8 NeuronCores.

Sharding: data-parallel over (batch, sequence-half) -> 8 shards, one per core.
Each core computes attention for 1024 query tokens of one batch element:
  - K/V projections over the full 2048-token sequence of its batch element
    (duplicated across the 2 cores sharing a batch element -- cheaper than
    communicating K/V, so the kernel needs no collectives),
  - Q projection for its 1024 queries,
  - per-head S^T = K_h @ Q_h^T, softmax over keys via exp + ones-column
    denominator folded into the A@V matmul,
  - output projection + residual + LayerNorm on its 1024 tokens.

Pipeline: V projection runs first, then per head-pair m the Q/K projections for
that pair interleave with QK^T + exp + A@V of the previous pair, so the ScalarE
exp stream (the phase-2 bottleneck) overlaps TensorE projection work instead of
serializing after it. Q^T/K^T slices are streamed per head-pair. LayerNorm's
gamma/beta affine runs on the otherwise-idle GpSimd engine.

All matmuls are bf16 inputs with fp32 PSUM accumulation; softmax statistics,
residual and LayerNorm stay fp32.
"""

import os
import sys

sys.path.insert(0, "/opt/trn_rl_repo")

import numpy as np

B, S, D, H = 4, 2048, 1024, 16
HD = D // H  # 64
SQ = S // 2  # queries per core
NCORES = 8
EPS = 1e-12

_CACHE = {}


def _install_ntff_hook():
    """Register the axon NTFF profile hook that bass_utils looks up via
    antenv.axon_hooks (absent from the image's antenv stub)."""
    import contextlib
    import ctypes
    import types

    so_path = "/opt/axon/libaxon_pjrt.so"
    if "antenv.axon_hooks" in sys.modules:
        return
    try:
        lib = ctypes.CDLL(so_path)
    except OSError:
        return
    if not hasattr(lib, "axon_start_nrt_profile"):
        return
    lib.axon_start_nrt_profile.argtypes = [ctypes.POINTER(ctypes.c_int64), ctypes.c_size_t]
    lib.axon_start_nrt_profile.restype = ctypes.c_int64
    lib.axon_stop_nrt_profile.argtypes = [ctypes.c_char_p]
    lib.axon_stop_nrt_profile.restype = ctypes.c_int64

    @contextlib.contextmanager
    def _hook(output_dir, device_ids):
        import jax

        jax.devices()
        if device_ids:
            ids = (ctypes.c_int64 * len(device_ids))(*device_ids)
            rc = lib.axon_start_nrt_profile(ids, len(device_ids))
        else:
            rc = lib.axon_start_nrt_profile(None, 0)
        if rc != 0:
            raise RuntimeError(f"axon_start_nrt_profile rc={rc}")
        try:
            yield
        finally:
            n = lib.axon_stop_nrt_profile(str(output_dir).encode())
            if n < 0:
                raise RuntimeError(f"axon_stop_nrt_profile rc={n}")

    m = types.ModuleType("antenv.axon_hooks")
    m.get_axon_ntff_profile_hook = lambda: _hook
    m.set_axon_ntff_profile_hook = lambda h: None
    sys.modules["antenv.axon_hooks"] = m


def _build():
    import concourse.bass as bass
    import concourse.tile as tile
    from concourse import bacc, mybir

    f32 = mybir.dt.float32
    bf16 = mybir.dt.bfloat16
    ADD = mybir.AluOpType.add
    MULT = mybir.AluOpType.mult
    SUB = mybir.AluOpType.subtract
    Exp = mybir.ActivationFunctionType.Exp
    Sqrt = mybir.ActivationFunctionType.Sqrt

    nc = bacc.Bacc("TRN2")

    xt_d = nc.dram_tensor("xt", [D, S], bf16, kind="ExternalInput")
    xq_d = nc.dram_tensor("xq", [SQ, D], f32, kind="ExternalInput")
    wq_d = nc.dram_tensor("wqt", [D, D], bf16, kind="ExternalInput")
    wk_d = nc.dram_tensor("wkt", [D, D], bf16, kind="ExternalInput")
    wv_d = nc.dram_tensor("wvt", [D, D], bf16, kind="ExternalInput")
    wo_d = nc.dram_tensor("wot", [D, D], bf16, kind="ExternalInput")
    bq_d = nc.dram_tensor("bqt", [128, 8], f32, kind="ExternalInput")
    bk_d = nc.dram_tensor("bkt", [128, 8], f32, kind="ExternalInput")
    bv_d = nc.dram_tensor("bv", [D], f32, kind="ExternalInput")
    bo_d = nc.dram_tensor("bo", [D], f32, kind="ExternalInput")
    gamma_d = nc.dram_tensor("gamma", [D], f32, kind="ExternalInput")
    beta_d = nc.dram_tensor("beta", [D], f32, kind="ExternalInput")
    sel2_d = nc.dram_tensor("sel2", [2, 128], bf16, kind="ExternalInput")
    out_d = nc.dram_tensor("out", [SQ, D], f32, kind="ExternalOutput")

    def bcast_ap(handle):
        ap = handle[:]
        return bass.AP(tensor=ap.tensor, offset=ap.offset, ap=[[0, 128], ap.ap[0]])

    with tile.TileContext(nc) as tc:
        with (
            tc.tile_pool(name="const", bufs=1) as constp,
            tc.tile_pool(name="v", bufs=1) as vp,
            tc.tile_pool(name="ot", bufs=1) as otp,
            tc.tile_pool(name="xt", bufs=1) as xtp,
        ):
            # --- constants ---
            bq_c = constp.tile([128, 8], f32, tag="bq")
            bk_c = constp.tile([128, 8], f32, tag="bk")
            bv_c = constp.tile([128, D], f32, tag="bv")
            bo_c = constp.tile([128, D], f32, tag="bo")
            gamma_c = constp.tile([128, D], f32, tag="gamma")
            beta_c = constp.tile([128, D], f32, tag="beta")
            eps_c = constp.tile([128, 1], f32, tag="eps")
            sel2_c = constp.tile([2, 128], bf16, tag="sel2")
            nc.sync.dma_start(out=bq_c[:], in_=bq_d[:])
            nc.sync.dma_start(out=bk_c[:], in_=bk_d[:])
            nc.gpsimd.dma_start(out=bv_c[:], in_=bcast_ap(bv_d))
            nc.gpsimd.dma_start(out=bo_c[:], in_=bcast_ap(bo_d))
            nc.gpsimd.dma_start(out=gamma_c[:], in_=bcast_ap(gamma_d))
            nc.gpsimd.dma_start(out=beta_c[:], in_=bcast_ap(beta_d))
            nc.sync.dma_start(out=sel2_c[:], in_=sel2_d[:])
            nc.vector.memset(eps_c[:], EPS)

            # --- persistent activations ---
            v = vp.tile([128, 16, H, HD + 1], bf16, tag="v")  # V + ones col
            ot = otp.tile([128, 8, SQ], bf16, tag="ot")       # O^T
            xt = xtp.tile([128, 8, S], bf16, tag="xt")

            nc.vector.memset(v[:, :, :, HD : HD + 1], 1.0)
            for r in range(8):
                nc.sync.dma_start(out=xt[:, r, :], in_=xt_d[r * 128 : (r + 1) * 128, :])

            with (
                tc.tile_pool(name="wvr", bufs=1) as wvrp,
                tc.tile_pool(name="qkw", bufs=2) as qkwp,
                tc.tile_pool(name="qts", bufs=2) as qtsp,
                tc.tile_pool(name="kts", bufs=2) as ktsp,
                tc.tile_pool(name="st", bufs=6) as stp,
                tc.tile_pool(name="stage", bufs=2) as stagep,
                tc.tile_pool(name="dens", bufs=1) as densp,
                tc.tile_pool(name="ps1", bufs=2, space="PSUM") as ps1,
                tc.tile_pool(name="sp", bufs=2, space="PSUM") as spp,
                tc.tile_pool(name="av", bufs=1, space="PSUM") as avp,
            ):
                # ---------- V projection (first: A@V consumes all of it) ----------
                wv_r = wvrp.tile([128, 8, D], bf16, tag="wvr")
                for k in range(8):
                    nc.sync.dma_start(
                        out=wv_r[:, k, :], in_=wv_d[k * 128 : (k + 1) * 128, :]
                    )
                for tc_i in range(16):
                    for dg in range(2):
                        psv = ps1.tile([128, 512], f32, tag="ps", name="psv")
                        for k in range(8):
                            nc.tensor.matmul(
                                out=psv[:],
                                lhsT=xt[:, k, tc_i * 128 : (tc_i + 1) * 128],
                                rhs=wv_r[:, k, dg * 512 : (dg + 1) * 512],
                                start=(k == 0),
                                stop=(k == 7),
                            )
                        nc.vector.tensor_tensor(
                            out=v[:, tc_i, dg * 8 : (dg + 1) * 8, 0:HD],
                            in0=psv[:].rearrange("p (h d) -> p h d", d=HD),
                            in1=bv_c[:, dg * 512 : (dg + 1) * 512].rearrange(
                                "p (h d) -> p h d", d=HD
                            ),
                            op=ADD,
                        )

                # ---------- interleaved: per head-pair m ----------
                # emit block m:  Q(m), K(m) projections; QK^T+exp heads 2m,2m+1;
                # then A@V + evac + normalize for the previous pair (lag 1).
                def proj_block(m):
                    wq_m = qkwp.tile([128, 8, 128], bf16, tag="qkw", name="wq_m")
                    nc.sync.dma_start(
                        out=wq_m[:],
                        in_=wq_d[:, m * 128 : (m + 1) * 128].rearrange(
                            "(k p) c -> p k c", p=128
                        ),
                    )
                    qt_m = qtsp.tile([128, SQ], bf16, tag="qts", name="qt_m")
                    for tg in range(2):
                        psq = ps1.tile([128, 512], f32, tag="ps", name="psq")
                        for k in range(8):
                            nc.tensor.matmul(
                                out=psq[:],
                                lhsT=wq_m[:, k, :],
                                rhs=xt[:, k, tg * 512 : (tg + 1) * 512],
                                start=(k == 0),
                                stop=(k == 7),
                            )
                        nc.vector.tensor_scalar(
                            out=qt_m[:, tg * 512 : (tg + 1) * 512],
                            in0=psq[:],
                            scalar1=bq_c[:, m : m + 1],
                            scalar2=None,
                            op0=ADD,
                        )
                    wk_m = qkwp.tile([128, 8, 128], bf16, tag="qkw", name="wk_m")
                    nc.sync.dma_start(
                        out=wk_m[:],
                        in_=wk_d[:, m * 128 : (m + 1) * 128].rearrange(
                            "(k p) c -> p k c", p=128
                        ),
                    )
                    kt_m = ktsp.tile([128, S], bf16, tag="kts", name="kt_m")
                    for tg in range(4):
                        psk = ps1.tile([128, 512], f32, tag="ps", name="psk")
                        for k in range(8):
                            nc.tensor.matmul(
                                out=psk[:],
                                lhsT=wk_m[:, k, :],
                                rhs=xt[:, k, tg * 512 : (tg + 1) * 512],
                                start=(k == 0),
                                stop=(k == 7),
                            )
                        nc.vector.tensor_scalar(
                            out=kt_m[:, tg * 512 : (tg + 1) * 512],
                            in0=psk[:],
                            scalar1=bk_c[:, m : m + 1],
                            scalar2=None,
                            op0=ADD,
                        )
                    return qt_m, kt_m

                def qk_exp_block(m, qt_m, kt_m):
                    st_pair = []
                    for hh in range(2):  # heads 2m, 2m+1
                        p0 = hh * 64
                        st_tiles = [
                            stp.tile([128, 4, SQ], bf16, tag="st", name="st")
                            for _ in range(4)
                        ]
                        for kc in range(16):
                            sp = spp.tile([128, 1024], f32, tag="sp", name="sp")
                            for qh in range(2):
                                nc.tensor.matmul(
                                    out=sp[:, qh * 512 : (qh + 1) * 512],
                                    lhsT=kt_m[p0 : p0 + 64, kc * 128 : (kc + 1) * 128],
                                    rhs=qt_m[p0 : p0 + 64, qh * 512 : (qh + 1) * 512],
                                    start=True,
                                    stop=True,
                                )
                            nc.scalar.activation(
                                out=st_tiles[kc // 4][:, kc % 4, :],
                                in_=sp[:],
                                func=Exp,
                                scale=float(1.0 / np.sqrt(HD)),
                            )
                        st_pair.append(st_tiles)
                    return st_pair

                def av_block(m, st_pair):
                    den_m = densp.tile([2, SQ], f32, tag="den", name="den_m")
                    for hh in range(2):
                        h = 2 * m + hh
                        st_tiles = st_pair[hh]
                        av = avp.tile([128, 1024], f32, tag="av", name="av")
                        for qh in range(2):
                            for kc in range(16):
                                nc.tensor.matmul(
                                    out=av[0:65, qh * 512 : (qh + 1) * 512],
                                    lhsT=v[:, kc, h, :],
                                    rhs=st_tiles[kc // 4][
                                        :, kc % 4, qh * 512 : (qh + 1) * 512
                                    ],
                                    start=(kc == 0),
                                    stop=(kc == 15),
                                )
                        stg_d = stagep.tile([65, SQ], f32, tag="stg_d", name="stg_d")
                        nc.vector.tensor_copy(out=stg_d[64:65, :], in_=av[64:65, :])
                        nc.sync.dma_start(out=den_m[hh : hh + 1, :], in_=stg_d[64:65, :])
                        if hh == 0:
                            nc.vector.tensor_copy(out=ot[0:64, m, :], in_=av[0:64, :])
                        else:
                            stg_o = stagep.tile([64, SQ], bf16, tag="stg_o", name="stg_o")
                            nc.vector.tensor_copy(out=stg_o[:, :], in_=av[0:64, :])
                            nc.sync.dma_start(out=ot[64:128, m, :], in_=stg_o[:, :])
                    # normalize: ot[:, m, :] *= 1/den broadcast over the head dims
                    rc_f = densp.tile([2, SQ], f32, tag="rcf", name="rc_f")
                    rc_b = densp.tile([2, SQ], bf16, tag="rcb", name="rc_b")
                    nc.vector.reciprocal(out=rc_f[:], in_=den_m[:])
                    nc.vector.tensor_copy(out=rc_b[:], in_=rc_f[:])
                    bc = avp.tile([128, 1024], f32, tag="av", name="bc")
                    for g in range(2):
                        nc.tensor.matmul(
                            out=bc[:, g * 512 : (g + 1) * 512],
                            lhsT=sel2_c[:],
                            rhs=rc_b[:, g * 512 : (g + 1) * 512],
                            start=True,
                            stop=True,
                        )
                    nc.vector.tensor_tensor(out=ot[:, m, :], in0=ot[:, m, :], in1=bc[:], op=MULT)

                prev = None
                for m in range(8):
                    qt_m, kt_m = proj_block(m)
                    st_pair = qk_exp_block(m, qt_m, kt_m)
                    if prev is not None:
                        av_block(prev[0], prev[1])
                    prev = (m, st_pair)
                av_block(prev[0], prev[1])

            # ========== phase 3: output projection + residual + LN ==========
            with (
                tc.tile_pool(name="wo", bufs=8) as wop,
                tc.tile_pool(name="xqp", bufs=8) as xqp,
                tc.tile_pool(name="xqbo", bufs=8) as xqbop,
                tc.tile_pool(name="y", bufs=3) as yp,
                tc.tile_pool(name="y2", bufs=3) as y2p,
                tc.tile_pool(name="stats", bufs=4) as statp,
                tc.tile_pool(name="ps3", bufs=4, space="PSUM") as ps3,
            ):
                wo_tiles = []
                for k in range(8):
                    wt = wop.tile([128, D], bf16, tag="wo", name="wo_t")
                    nc.sync.dma_start(out=wt[:], in_=wo_d[k * 128 : (k + 1) * 128, :])
                    wo_tiles.append(wt)
                xq_tiles = []
                for t in range(8):
                    xq_t = xqp.tile([128, D], f32, tag="xq", name="xq_t")
                    nc.sync.dma_start(out=xq_t[:], in_=xq_d[t * 128 : (t + 1) * 128, :])
                    xq_tiles.append(xq_t)
                xqbo_tiles = []
                for t in range(8):
                    xqbo = xqbop.tile([128, D], f32, tag="xqbo", name="xqbo_t")
                    nc.gpsimd.tensor_tensor(
                        out=xqbo[:], in0=xq_tiles[t][:], in1=bo_c[:], op=ADD
                    )
                    xqbo_tiles.append(xqbo)
                for tg in range(2):
                    ps = [ps3.tile([128, 1024], f32, tag="ps", name="ps3") for _ in range(4)]
                    for k in range(8):
                        wt = wo_tiles[k]
                        for i in range(4):
                            t = tg * 4 + i
                            for g in range(2):
                                nc.tensor.matmul(
                                    out=ps[i][:, g * 512 : (g + 1) * 512],
                                    lhsT=ot[:, k, t * 128 : (t + 1) * 128],
                                    rhs=wt[:, g * 512 : (g + 1) * 512],
                                    start=(k == 0),
                                    stop=(k == 7),
                                )
                    for i in range(4):
                        t = tg * 4 + i
                        xqbo = xqbo_tiles[t]
                        y = yp.tile([128, D], f32, tag="y")
                        nc.vector.tensor_tensor(out=y[:], in0=ps[i][:], in1=xqbo[:], op=ADD)
                        stats = statp.tile([128, 2, 6], f32, tag="stats")
                        mv = statp.tile([128, 2], f32, tag="mv")
                        nc.vector.bn_stats(out=stats[:, 0, :], in_=y[:, 0:512])
                        nc.vector.bn_stats(out=stats[:, 1, :], in_=y[:, 512:1024])
                        nc.vector.bn_aggr(out=mv[:], in_=stats[:])
                        nc.scalar.activation(
                            out=mv[:, 1:2], in_=mv[:, 1:2], func=Sqrt, bias=eps_c[:, 0:1]
                        )
                        nc.vector.reciprocal(out=mv[:, 1:2], in_=mv[:, 1:2])
                        nc.vector.tensor_scalar(
                            out=y[:],
                            in0=y[:],
                            scalar1=mv[:, 0:1],
                            scalar2=mv[:, 1:2],
                            op0=SUB,
                            op1=MULT,
                        )
                        y2 = y2p.tile([128, D], f32, tag="y2")
                        nc.vector.tensor_tensor(out=y2[:], in0=y[:], in1=gamma_c[:], op=MULT)
                        nc.gpsimd.tensor_tensor(out=y2[:], in0=y2[:], in1=beta_c[:], op=ADD)
                        nc.sync.dma_start(out=out_d[t * 128 : (t + 1) * 128, :], in_=y2[:])

    nc.compile()
    return nc


def _get_nc():
    if "nc" not in _CACHE:
        _CACHE["nc"] = _build()
    return _CACHE["nc"]


def kernel(X, Wq, bq, Wk, bk, Wv, bv, Wo, bo, gamma, beta):
    if os.environ.get("BASS_TRACE"):
        _install_ntff_hook()
    import ml_dtypes

    from concourse.bass_utils import run_bass_kernel_spmd

    bfdt = ml_dtypes.bfloat16
    f32 = np.float32
    X = np.ascontiguousarray(np.asarray(X, dtype=f32))
    wqt = np.ascontiguousarray(np.asarray(Wq, f32).T.astype(bfdt))
    wkt = np.ascontiguousarray(np.asarray(Wk, f32).T.astype(bfdt))
    wvt = np.ascontiguousarray(np.asarray(Wv, f32).T.astype(bfdt))
    wot = np.ascontiguousarray(np.asarray(Wo, f32).T.astype(bfdt))
    bqt = np.ascontiguousarray(np.asarray(bq, f32).reshape(8, 128).T)
    bkt = np.ascontiguousarray(np.asarray(bk, f32).reshape(8, 128).T)
    bv_ = np.ascontiguousarray(np.asarray(bv, f32))
    bo_ = np.ascontiguousarray(np.asarray(bo, f32))
    gamma_ = np.ascontiguousarray(np.asarray(gamma, f32))
    beta_ = np.ascontiguousarray(np.asarray(beta, f32))
    sel2 = np.zeros((2, 128), f32)
    sel2[0, 0:64] = 1.0
    sel2[1, 64:128] = 1.0
    sel2 = sel2.astype(bfdt)

    in_maps = []
    for c in range(NCORES):
        b, half = c // 2, c % 2
        Xb = X[b]
        q_rows = Xb[half * SQ : (half + 1) * SQ]
        o_rows = Xb[(1 - half) * SQ : (2 - half) * SQ]
        # queries-first token order (key order is permutation-invariant)
        xt = np.ascontiguousarray(np.concatenate([q_rows, o_rows], axis=0).T.astype(bfdt))
        in_maps.append(
            {
                "xt": xt,
                "xq": np.ascontiguousarray(q_rows),
                "wqt": wqt,
                "wkt": wkt,
                "wvt": wvt,
                "wot": wot,
                "bqt": bqt,
                "bkt": bkt,
                "bv": bv_,
                "bo": bo_,
                "gamma": gamma_,
                "beta": beta_,
                "sel2": sel2,
            }
        )

    nc = _get_nc()
    res = run_bass_kernel_spmd(nc, in_maps, core_ids=list(range(NCORES)))
    if res.exec_time_ns is not None:
        print(f"HW exec time: {res.exec_time_ns} ns")

    out = np.empty((B, S, D), np.float32)
    for c in range(NCORES):
        b, half = c // 2, c % 2
        out[b, half * SQ : (half + 1) * SQ] = res.results[c]["out"]
    return out



# revision 3
# speedup vs baseline: 1.2814x; 1.2814x over previous
"""Trainium2 Bass kernel for nn_Attention (B=4, S=2048, D=1024, H=16) on 8 NeuronCores.

Sharding: data-parallel over (batch, sequence-half) -> 8 shards, one per core.
Each core computes attention for 1024 query tokens of one batch element.

v2: fp8 rework of the bf16 baseline.
 - All projections (Q/K/V/O) run as fp8e4m3 DoubleRow matmuls: contraction of
   2x128 partitions per instruction at the same issue rate as bf16, i.e. 2x
   the throughput.
 - QK^T stays bf16 but alternates the 64-partition stationary base between
   head 0 (partitions 0:64) and head 1 (64:128) of each pair, so the PE
   loads one head's keys while streaming the other's queries (~2x issue rate).
 - Attention weights A = exp(s/8 - 4?) are stored in fp8e5m2 (wide dynamic
   range; raw scores reach 73 > e4m3 budget). The exp stream - the biggest
   elementwise cost - is split between ScalarE (exact exp, fp8 output) and
   DVE (Schraudolph: affine in log2 domain + saturating round to uint8 IS the
   fp8e5m2 bit pattern). A@V contracts A (e5m2) against V (e4m3) in DoubleRow
   mode with a ones-column folding the softmax denominator into the matmul.
 - Denominators collect into [8,S] tiles; batched reciprocal + per-pair
   broadcast matmul (sel8) normalizes O^T, written fp8 for the O projection.
 - Residual + LayerNorm in fp32; the (x-mu)*rstd affine runs on ScalarE with
   per-partition scale/bias APs.
Zero biases / identity gamma,beta (checked on host) skip their instructions.
"""

import os
import sys

sys.path.insert(0, "/opt/trn_rl_repo")

import numpy as np

B, S, D, H = 4, 2048, 1024, 16
HD = D // H  # 64
SQ = S // 2  # queries per core
NCORES = 8
EPS = 1e-12

SHIFT = 2.5
LOG2E = 1.4426950408889634
A_DVE = LOG2E * 0.125 * 4               # schraudolph slope per raw score (e5m2)
B_DVE = 60.0 - 0.225 - 4 * SHIFT * LOG2E  # fitted offset (c=-0.225)

_CACHE = {}


def _install_ntff_hook():
    """Register the axon NTFF profile hook that bass_utils looks up via
    antenv.axon_hooks (absent from the image's antenv stub)."""
    import contextlib
    import ctypes
    import types

    so_path = "/opt/axon/libaxon_pjrt.so"
    if "antenv.axon_hooks" in sys.modules:
        return
    try:
        lib = ctypes.CDLL(so_path)
    except OSError:
        return
    if not hasattr(lib, "axon_start_nrt_profile"):
        return
    lib.axon_start_nrt_profile.argtypes = [ctypes.POINTER(ctypes.c_int64), ctypes.c_size_t]
    lib.axon_start_nrt_profile.restype = ctypes.c_int64
    lib.axon_stop_nrt_profile.argtypes = [ctypes.c_char_p]
    lib.axon_stop_nrt_profile.restype = ctypes.c_int64

    @contextlib.contextmanager
    def _hook(output_dir, device_ids):
        import jax

        jax.devices()
        if device_ids:
            ids = (ctypes.c_int64 * len(device_ids))(*device_ids)
            rc = lib.axon_start_nrt_profile(ids, len(device_ids))
        else:
            rc = lib.axon_start_nrt_profile(None, 0)
        if rc != 0:
            raise RuntimeError(f"axon_start_nrt_profile rc={rc}")
        try:
            yield
        finally:
            n = lib.axon_stop_nrt_profile(str(output_dir).encode())
            if n < 0:
                raise RuntimeError(f"axon_stop_nrt_profile rc={n}")

    m = types.ModuleType("antenv.axon_hooks")
    m.get_axon_ntff_profile_hook = lambda: _hook
    m.set_axon_ntff_profile_hook = lambda h: None
    sys.modules["antenv.axon_hooks"] = m


def _build(flags):
    use_bq, use_bk, use_bv, use_bo, use_gamma, use_beta = flags

    import concourse.bass as bass
    import concourse.tile as tile
    from concourse import bacc, mybir

    f32 = mybir.dt.float32
    bf16 = mybir.dt.bfloat16
    fp8 = mybir.dt.float8e4
    fp8e5 = mybir.dt.float8e5
    u8 = mybir.dt.uint8
    ADD = mybir.AluOpType.add
    MULT = mybir.AluOpType.mult
    SUB = mybir.AluOpType.subtract
    Exp = mybir.ActivationFunctionType.Exp
    Sqrt = mybir.ActivationFunctionType.Sqrt
    Copy = mybir.ActivationFunctionType.Copy
    Ident = mybir.ActivationFunctionType.Identity
    DR = mybir.MatmulPerfMode.DoubleRow

    nc = bacc.Bacc("TRN2")

    xt_d = nc.dram_tensor("xt", [D, S], fp8, kind="ExternalInput")
    xq_d = nc.dram_tensor("xq", [SQ, D], f32, kind="ExternalInput")
    wq_d = nc.dram_tensor("wqt", [D, D], fp8, kind="ExternalInput")
    wk_d = nc.dram_tensor("wkt", [D, D], fp8, kind="ExternalInput")
    wv_d = nc.dram_tensor("wvt", [D, D], fp8, kind="ExternalInput")
    wo_d = nc.dram_tensor("wot", [D, D], fp8, kind="ExternalInput")
    bq_d = nc.dram_tensor("bqt", [128, 8], f32, kind="ExternalInput")
    bk_d = nc.dram_tensor("bkt", [128, 8], f32, kind="ExternalInput")
    bv_d = nc.dram_tensor("bv", [D], f32, kind="ExternalInput")
    bo_d = nc.dram_tensor("bo", [D], f32, kind="ExternalInput")
    gamma_d = nc.dram_tensor("gamma", [D], f32, kind="ExternalInput")
    beta_d = nc.dram_tensor("beta", [D], f32, kind="ExternalInput")
    sel8_d = nc.dram_tensor("sel8", [8, 4, 128], bf16, kind="ExternalInput")
    out_d = nc.dram_tensor("out", [SQ, D], f32, kind="ExternalOutput")

    def bcast_ap(handle):
        ap = handle[:]
        return bass.AP(tensor=ap.tensor, offset=ap.offset, ap=[[0, 128], ap.ap[0]])

    # which (hh, kc) score tiles go to ScalarE (exact exp) vs DVE (schraudolph)
    scalar_set = {i for i in range(32) if (i * 17) % 32 < 17}

    with tile.TileContext(nc) as tc:
        with (
            tc.tile_pool(name="const", bufs=1) as constp,
            tc.tile_pool(name="v", bufs=1) as vp,
            tc.tile_pool(name="ot", bufs=1) as otp,
            tc.tile_pool(name="xt", bufs=1) as xtp,
            tc.tile_pool(name="wo", bufs=1) as wop,
        ):
            # --- constants ---
            bq_c = constp.tile([128, 8], f32, tag="bq")
            bk_c = constp.tile([128, 8], f32, tag="bk")
            bv_c = constp.tile([128, D], f32, tag="bv")
            gamma_c = constp.tile([128, D], f32, tag="gamma")
            beta_c = constp.tile([128, D], f32, tag="beta")
            bo_c = constp.tile([128, D], f32, tag="bo")
            eps_c = constp.tile([128, 1], f32, tag="eps")
            nshift_c = constp.tile([128, 1], f32, tag="nshift")
            sel8_c = constp.tile([8, 4, 128], bf16, tag="sel8")
            if use_bq:
                nc.sync.dma_start(out=bq_c[:], in_=bq_d[:])
            if use_bk:
                nc.sync.dma_start(out=bk_c[:], in_=bk_d[:])
            if use_bv:
                nc.gpsimd.dma_start(out=bv_c[:], in_=bcast_ap(bv_d))
            if use_bo:
                nc.gpsimd.dma_start(out=bo_c[:], in_=bcast_ap(bo_d))
            if use_gamma:
                nc.gpsimd.dma_start(out=gamma_c[:], in_=bcast_ap(gamma_d))
            if use_beta:
                nc.gpsimd.dma_start(out=beta_c[:], in_=bcast_ap(beta_d))
            nc.sync.dma_start(out=sel8_c[:], in_=sel8_d[:])
            nc.vector.memset(eps_c[:], EPS)
            nc.vector.memset(nshift_c[:], -SHIFT)

            # --- persistent activations ---
            v8 = vp.tile([128, 16, H, HD + 1], fp8, tag="v")   # V + ones col (den)
            otb = otp.tile([128, 8, SQ], bf16, tag="otb")      # O^T unnormalized
            ot8 = otp.tile([128, 8, SQ], fp8, tag="ot8")       # O^T normalized
            den_a = otp.tile([8, SQ], bf16, tag="den_a")       # heads 0-7
            den_b = otp.tile([8, SQ], bf16, tag="den_b")       # heads 8-15
            xt = xtp.tile([128, 8, S], fp8, tag="xt")
            wo_r = wop.tile([128, 8, D], fp8, tag="wor")

            nc.vector.memset(v8[:, :, :, HD : HD + 1], 1.0)
            for r in range(8):
                nc.sync.dma_start(out=xt[:, r, :], in_=xt_d[r * 128 : (r + 1) * 128, :])
                nc.gpsimd.dma_start(out=wo_r[:, r, :], in_=wo_d[r * 128 : (r + 1) * 128, :])

            with (
                tc.tile_pool(name="wvr", bufs=1) as wvrp,
                tc.tile_pool(name="qkw", bufs=2) as qkwp,
                tc.tile_pool(name="qts", bufs=2) as qtsp,
                tc.tile_pool(name="kts", bufs=2) as ktsp,
                tc.tile_pool(name="st", bufs=12) as stp,
                tc.tile_pool(name="stage", bufs=4) as stagep,
                tc.tile_pool(name="rc", bufs=2) as rcp,
                tc.tile_pool(name="ps1", bufs=2, space="PSUM") as ps1,
                tc.tile_pool(name="sp", bufs=2, space="PSUM") as spp,
                tc.tile_pool(name="av", bufs=2, space="PSUM") as avp,
            ):
                # ---------- V projection (fp8 DoubleRow) ----------
                wv_r = wvrp.tile([128, 8, D], fp8, tag="wvr")
                for k in range(8):
                    nc.sync.dma_start(
                        out=wv_r[:, k, :], in_=wv_d[k * 128 : (k + 1) * 128, :]
                    )
                for tc_i in range(16):
                    for dg in range(2):
                        psv = ps1.tile([128, 512], f32, tag="ps", name="psv")
                        for k in range(4):
                            nc.tensor.matmul(
                                out=psv[:],
                                lhsT=xt[:, 2 * k : 2 * k + 2, tc_i * 128 : (tc_i + 1) * 128],
                                rhs=wv_r[:, 2 * k : 2 * k + 2, dg * 512 : (dg + 1) * 512],
                                start=(k == 0),
                                stop=(k == 3),
                                perf_mode=DR,
                            )
                        dst = v8[:, tc_i, dg * 8 : (dg + 1) * 8, 0:HD]
                        if use_bv:
                            nc.vector.tensor_tensor(
                                out=dst,
                                in0=psv[:].rearrange("p (h d) -> p h d", d=HD),
                                in1=bv_c[:, dg * 512 : (dg + 1) * 512].rearrange(
                                    "p (h d) -> p h d", d=HD
                                ),
                                op=ADD,
                            )
                        else:
                            nc.vector.tensor_copy(
                                out=dst, in_=psv[:].rearrange("p (h d) -> p h d", d=HD)
                            )

                # ---------- per head-pair pipeline ----------
                def proj_block(m):
                    wq_m = qkwp.tile([128, 8, 128], fp8, tag="qkw", name="wq_m")
                    nc.sync.dma_start(
                        out=wq_m[:],
                        in_=wq_d[:, m * 128 : (m + 1) * 128].rearrange(
                            "(k p) c -> p k c", p=128
                        ),
                    )
                    qt_m = qtsp.tile([128, SQ], bf16, tag="qts", name="qt_m")
                    for tg in range(2):
                        psq = ps1.tile([128, 512], f32, tag="ps", name="psq")
                        for k in range(4):
                            nc.tensor.matmul(
                                out=psq[:],
                                lhsT=wq_m[:, 2 * k : 2 * k + 2, :],
                                rhs=xt[:, 2 * k : 2 * k + 2, tg * 512 : (tg + 1) * 512],
                                start=(k == 0),
                                stop=(k == 3),
                                perf_mode=DR,
                            )
                        if use_bq:
                            nc.scalar.activation(
                                out=qt_m[:, tg * 512 : (tg + 1) * 512],
                                in_=psq[:],
                                func=Ident,
                                bias=bq_c[:, m : m + 1],
                            )
                        else:
                            nc.scalar.copy(
                                out=qt_m[:, tg * 512 : (tg + 1) * 512], in_=psq[:]
                            )
                    wk_m = qkwp.tile([128, 8, 128], fp8, tag="qkw", name="wk_m")
                    nc.sync.dma_start(
                        out=wk_m[:],
                        in_=wk_d[:, m * 128 : (m + 1) * 128].rearrange(
                            "(k p) c -> p k c", p=128
                        ),
                    )
                    kt_m = ktsp.tile([128, S], bf16, tag="kts", name="kt_m")
                    for tg in range(4):
                        psk = ps1.tile([128, 512], f32, tag="ps", name="psk")
                        for k in range(4):
                            nc.tensor.matmul(
                                out=psk[:],
                                lhsT=wk_m[:, 2 * k : 2 * k + 2, :],
                                rhs=xt[:, 2 * k : 2 * k + 2, tg * 512 : (tg + 1) * 512],
                                start=(k == 0),
                                stop=(k == 3),
                                perf_mode=DR,
                            )
                        if use_bk:
                            nc.scalar.activation(
                                out=kt_m[:, tg * 512 : (tg + 1) * 512],
                                in_=psk[:],
                                func=Ident,
                                bias=bk_c[:, m : m + 1],
                            )
                        else:
                            nc.scalar.copy(
                                out=kt_m[:, tg * 512 : (tg + 1) * 512], in_=psk[:]
                            )
                    return qt_m, kt_m

                def qk_exp_block(m, qt_m, kt_m):
                    st_pair = [
                        [stp.tile([128, 4, SQ], fp8e5, tag="st", name="st") for _ in range(4)]
                        for _ in range(2)
                    ]
                    for kc in range(16):
                        sps = []
                        for hh in range(2):
                            sp = spp.tile([128, 1024], f32, tag="sp", name="sp")
                            sps.append(sp)
                        # interleave the two heads so the stationary base
                        # alternates 0/64 every matmul (ldweights overlaps)
                        for qh in range(2):
                            for hh in range(2):
                                p0 = hh * 64
                                nc.tensor.matmul(
                                    out=sps[hh][:, qh * 512 : (qh + 1) * 512],
                                    lhsT=kt_m[p0 : p0 + 64, kc * 128 : (kc + 1) * 128],
                                    rhs=qt_m[p0 : p0 + 64, qh * 512 : (qh + 1) * 512],
                                    start=True,
                                    stop=True,
                                )
                        for hh in range(2):
                            dst = st_pair[hh][kc // 4][:, kc % 4, :]
                            if (2 * kc + hh) in scalar_set:
                                nc.scalar.activation(
                                    out=dst,
                                    in_=sps[hh][:],
                                    func=Exp,
                                    scale=0.125,
                                    bias=nshift_c[:, 0:1],
                                )
                            else:
                                nc.vector.tensor_scalar(
                                    out=dst.bitcast(u8),
                                    in0=sps[hh][:],
                                    scalar1=float(A_DVE),
                                    scalar2=float(B_DVE),
                                    op0=MULT,
                                    op1=ADD,
                                )
                    return st_pair

                def av_block(m, st_pair):
                    den_t = den_a if m < 4 else den_b
                    for hh in range(2):
                        h = 2 * m + hh
                        st_tiles = st_pair[hh]
                        stg = stagep.tile([65, 2, 512], bf16, tag="stg", name="stg")
                        for qh in range(2):
                            av = avp.tile([128, 512], f32, tag="av", name="av")
                            for c in range(8):
                                u, j = c // 2, c % 2
                                nc.tensor.matmul(
                                    out=av[0:65, :],
                                    lhsT=v8[:, 4 * u + 2 * j : 4 * u + 2 * j + 2, h, :],
                                    rhs=st_tiles[u][:, 2 * j : 2 * j + 2, qh * 512 : (qh + 1) * 512],
                                    start=(c == 0),
                                    stop=(c == 7),
                                    perf_mode=DR,
                                )
                            if hh == 0:
                                nc.vector.tensor_copy(
                                    out=otb[0:64, m, qh * 512 : (qh + 1) * 512],
                                    in_=av[0:64, :],
                                )
                                nc.scalar.copy(out=stg[64:65, qh, :], in_=av[64:65, :])
                            else:
                                nc.scalar.copy(out=stg[0:65, qh, :], in_=av[0:65, :])
                        if hh == 0:
                            nc.sync.dma_start(
                                out=den_t[2 * (m % 4) : 2 * (m % 4) + 1, :],
                                in_=stg[64:65, :, :],
                            )
                        else:
                            nc.sync.dma_start(
                                out=otb[64:128, m, :], in_=stg[0:64, :, :]
                            )
                            nc.sync.dma_start(
                                out=den_t[2 * (m % 4) + 1 : 2 * (m % 4) + 2, :],
                                in_=stg[64:65, :, :],
                            )

                def norm_batch(b_i):
                    den_t = den_a if b_i == 0 else den_b
                    rc_f = rcp.tile([8, SQ], f32, tag="rcf", name="rc_f")
                    rc_b = rcp.tile([8, SQ], bf16, tag="rcb", name="rc_b")
                    nc.vector.reciprocal(out=rc_f[:], in_=den_t[:])
                    nc.vector.tensor_copy(out=rc_b[:], in_=rc_f[:])
                    for mm in range(4):
                        m = 4 * b_i + mm
                        for qh in range(2):
                            bc = ps1.tile([128, 512], f32, tag="ps", name="bc")
                            nc.tensor.matmul(
                                out=bc[:],
                                lhsT=sel8_c[:, mm, :],
                                rhs=rc_b[:, qh * 512 : (qh + 1) * 512],
                                start=True,
                                stop=True,
                            )
                            nc.vector.tensor_tensor(
                                out=ot8[:, m, qh * 512 : (qh + 1) * 512],
                                in0=otb[:, m, qh * 512 : (qh + 1) * 512],
                                in1=bc[:],
                                op=MULT,
                            )

                prev = None
                for m in range(8):
                    qt_m, kt_m = proj_block(m)
                    st_pair = qk_exp_block(m, qt_m, kt_m)
                    if prev is not None:
                        av_block(prev[0], prev[1])
                        if prev[0] == 3:
                            norm_batch(0)
                    prev = (m, st_pair)
                av_block(prev[0], prev[1])
                norm_batch(1)

            # ========== phase 3: O projection + residual + LN ==========
            with (
                tc.tile_pool(name="xqp", bufs=8) as xqp,
                tc.tile_pool(name="y", bufs=3) as yp,
                tc.tile_pool(name="yo", bufs=3) as yop,
                tc.tile_pool(name="stats", bufs=4) as statp,
                tc.tile_pool(name="ps3", bufs=2, space="PSUM") as ps3,
            ):
                xq_tiles = []
                for t in range(8):
                    xq_t = xqp.tile([128, D], f32, tag="xq", name="xq_t")
                    nc.sync.dma_start(out=xq_t[:], in_=xq_d[t * 128 : (t + 1) * 128, :])
                    xq_tiles.append(xq_t)
                if use_bo:
                    for t in range(8):
                        nc.gpsimd.tensor_tensor(
                            out=xq_tiles[t][:], in0=xq_tiles[t][:], in1=bo_c[:], op=ADD
                        )
                for t in range(8):
                    ps = ps3.tile([128, D], f32, tag="ps3", name="ps3")
                    for g in range(2):
                        for k in range(4):
                            nc.tensor.matmul(
                                out=ps[:, g * 512 : (g + 1) * 512],
                                lhsT=ot8[:, 2 * k : 2 * k + 2, t * 128 : (t + 1) * 128],
                                rhs=wo_r[:, 2 * k : 2 * k + 2, g * 512 : (g + 1) * 512],
                                start=(k == 0),
                                stop=(k == 3),
                                perf_mode=DR,
                            )
                    y = yp.tile([128, D], f32, tag="y")
                    nc.vector.tensor_tensor(out=y[:], in0=ps[:], in1=xq_tiles[t][:], op=ADD)
                    stats = statp.tile([128, 2, 6], f32, tag="stats")
                    mv = statp.tile([128, 2], f32, tag="mv")
                    mr = statp.tile([128, 1], f32, tag="mr")
                    nc.vector.bn_stats(out=stats[:, 0, :], in_=y[:, 0:512])
                    nc.vector.bn_stats(out=stats[:, 1, :], in_=y[:, 512:1024])
                    nc.vector.bn_aggr(out=mv[:], in_=stats[:])
                    nc.scalar.activation(
                        out=mv[:, 1:2], in_=mv[:, 1:2], func=Sqrt, bias=eps_c[:, 0:1]
                    )
                    nc.vector.reciprocal(out=mv[:, 1:2], in_=mv[:, 1:2])
                    nc.vector.tensor_scalar(
                        out=mr[:],
                        in0=mv[:, 0:1],
                        scalar1=mv[:, 1:2],
                        scalar2=-1.0,
                        op0=MULT,
                        op1=MULT,
                    )
                    yo = yop.tile([128, D], f32, tag="yo")
                    nc.scalar.activation(
                        out=yo[:], in_=y[:], func=Ident,
                        scale=mv[:, 1:2], bias=mr[:, 0:1],
                    )
                    if use_gamma:
                        nc.vector.tensor_tensor(out=yo[:], in0=yo[:], in1=gamma_c[:], op=MULT)
                    if use_beta:
                        nc.gpsimd.tensor_tensor(out=yo[:], in0=yo[:], in1=beta_c[:], op=ADD)
                    nc.sync.dma_start(out=out_d[t * 128 : (t + 1) * 128, :], in_=yo[:])

    nc.compile()
    return nc


def _get_nc(flags):
    key = ("nc", flags)
    if key not in _CACHE:
        _CACHE[key] = _build(flags)
    return _CACHE[key]


def kernel(X, Wq, bq, Wk, bk, Wv, bv, Wo, bo, gamma, beta):
    if os.environ.get("BASS_TRACE"):
        _install_ntff_hook()
    import ml_dtypes

    from concourse.bass_utils import run_bass_kernel_spmd

    f8 = ml_dtypes.float8_e4m3
    bfdt = ml_dtypes.bfloat16
    f32 = np.float32
    X = np.ascontiguousarray(np.asarray(X, dtype=f32))
    bq_ = np.asarray(bq, f32)
    bk_ = np.asarray(bk, f32)
    bv_ = np.ascontiguousarray(np.asarray(bv, f32))
    bo_ = np.ascontiguousarray(np.asarray(bo, f32))
    gamma_ = np.ascontiguousarray(np.asarray(gamma, f32))
    beta_ = np.ascontiguousarray(np.asarray(beta, f32))
    flags = (
        bool(np.any(bq_)), bool(np.any(bk_)), bool(np.any(bv_)), bool(np.any(bo_)),
        bool(np.any(gamma_ != 1.0)), bool(np.any(beta_)),
    )

    wqt = np.ascontiguousarray(np.asarray(Wq, f32).T.astype(f8))
    wkt = np.ascontiguousarray(np.asarray(Wk, f32).T.astype(f8))
    wvt = np.ascontiguousarray(np.asarray(Wv, f32).T.astype(f8))
    wot = np.ascontiguousarray(np.asarray(Wo, f32).T.astype(f8))
    bqt = np.ascontiguousarray(bq_.reshape(8, 128).T)
    bkt = np.ascontiguousarray(bk_.reshape(8, 128).T)
    sel8 = np.zeros((8, 4, 128), f32)
    for mm in range(4):
        for p in range(128):
            sel8[2 * mm + p // 64, mm, p] = 1.0
    sel8 = sel8.astype(bfdt)

    in_maps = []
    for c in range(NCORES):
        b, half = c // 2, c % 2
        Xb = X[b]
        q_rows = Xb[half * SQ : (half + 1) * SQ]
        o_rows = Xb[(1 - half) * SQ : (2 - half) * SQ]
        # queries-first token order (key order is permutation-invariant)
        xt = np.ascontiguousarray(np.concatenate([q_rows, o_rows], axis=0).T.astype(f8))
        in_maps.append(
            {
                "xt": xt,
                "xq": np.ascontiguousarray(q_rows),
                "wqt": wqt,
                "wkt": wkt,
                "wvt": wvt,
                "wot": wot,
                "bqt": bqt,
                "bkt": bkt,
                "bv": bv_,
                "bo": bo_,
                "gamma": gamma_,
                "beta": beta_,
                "sel8": sel8,
            }
        )

    nc = _get_nc(flags)
    res = run_bass_kernel_spmd(nc, in_maps, core_ids=list(range(NCORES)))
    if res.exec_time_ns is not None:
        print(f"HW exec time: {res.exec_time_ns} ns")

    out = np.empty((B, S, D), np.float32)
    for c in range(NCORES):
        b, half = c // 2, c % 2
        out[b, half * SQ : (half + 1) * SQ] = res.results[c]["out"]
    return out


# revision 5
# speedup vs baseline: 1.2945x; 1.0103x over previous
"""Trainium2 Bass kernel for nn_Attention (B=4, S=2048, D=1024, H=16) on 8 NeuronCores.

Sharding: data-parallel over (batch, sequence-half) -> 8 shards, one per core.
Each core computes attention for 1024 query tokens of one batch element.

v2: fp8 rework of the bf16 baseline.
 - All projections (Q/K/V/O) run as fp8e4m3 DoubleRow matmuls: contraction of
   2x128 partitions per instruction at the same issue rate as bf16, i.e. 2x
   the throughput.
 - QK^T stays bf16 but alternates the 64-partition stationary base between
   head 0 (partitions 0:64) and head 1 (64:128) of each pair, so the PE
   loads one head's keys while streaming the other's queries (~2x issue rate).
 - Attention weights A = exp(s/8 - 4?) are stored in fp8e5m2 (wide dynamic
   range; raw scores reach 73 > e4m3 budget). The exp stream - the biggest
   elementwise cost - is split between ScalarE (exact exp, fp8 output) and
   DVE (Schraudolph: affine in log2 domain + saturating round to uint8 IS the
   fp8e5m2 bit pattern). A@V contracts A (e5m2) against V (e4m3) in DoubleRow
   mode with a ones-column folding the softmax denominator into the matmul.
 - Denominators collect into [8,S] tiles; batched reciprocal + per-pair
   broadcast matmul (sel8) normalizes O^T, written fp8 for the O projection.
 - Residual + LayerNorm in fp32; the (x-mu)*rstd affine runs on ScalarE with
   per-partition scale/bias APs.
Zero biases / identity gamma,beta (checked on host) skip their instructions.
"""

import os
import sys

sys.path.insert(0, "/opt/trn_rl_repo")

import numpy as np

B, S, D, H = 4, 2048, 1024, 16
HD = D // H  # 64
SQ = S // 2  # queries per core
NCORES = 8
EPS = 1e-12

SHIFT = 2.5
LOG2E = 1.4426950408889634
A_DVE = LOG2E * 0.125 * 4               # schraudolph slope per raw score (e5m2)
B_DVE = 60.0 - 0.225 - 4 * SHIFT * LOG2E  # fitted offset (c=-0.225)

_CACHE = {}


def _install_ntff_hook():
    """Register the axon NTFF profile hook that bass_utils looks up via
    antenv.axon_hooks (absent from the image's antenv stub)."""
    import contextlib
    import ctypes
    import types

    so_path = "/opt/axon/libaxon_pjrt.so"
    if "antenv.axon_hooks" in sys.modules:
        return
    try:
        lib = ctypes.CDLL(so_path)
    except OSError:
        return
    if not hasattr(lib, "axon_start_nrt_profile"):
        return
    lib.axon_start_nrt_profile.argtypes = [ctypes.POINTER(ctypes.c_int64), ctypes.c_size_t]
    lib.axon_start_nrt_profile.restype = ctypes.c_int64
    lib.axon_stop_nrt_profile.argtypes = [ctypes.c_char_p]
    lib.axon_stop_nrt_profile.restype = ctypes.c_int64

    @contextlib.contextmanager
    def _hook(output_dir, device_ids):
        import jax

        jax.devices()
        if device_ids:
            ids = (ctypes.c_int64 * len(device_ids))(*device_ids)
            rc = lib.axon_start_nrt_profile(ids, len(device_ids))
        else:
            rc = lib.axon_start_nrt_profile(None, 0)
        if rc != 0:
            raise RuntimeError(f"axon_start_nrt_profile rc={rc}")
        try:
            yield
        finally:
            n = lib.axon_stop_nrt_profile(str(output_dir).encode())
            if n < 0:
                raise RuntimeError(f"axon_stop_nrt_profile rc={n}")

    m = types.ModuleType("antenv.axon_hooks")
    m.get_axon_ntff_profile_hook = lambda: _hook
    m.set_axon_ntff_profile_hook = lambda h: None
    sys.modules["antenv.axon_hooks"] = m


def _build(flags):
    use_bq, use_bk, use_bv, use_bo, use_gamma, use_beta = flags

    import concourse.bass as bass
    import concourse.tile as tile
    from concourse import bacc, mybir

    f32 = mybir.dt.float32
    bf16 = mybir.dt.bfloat16
    fp8 = mybir.dt.float8e4
    fp8e5 = mybir.dt.float8e5
    f32r = mybir.dt.float32r
    u8 = mybir.dt.uint8
    ADD = mybir.AluOpType.add
    MULT = mybir.AluOpType.mult
    SUB = mybir.AluOpType.subtract
    Exp = mybir.ActivationFunctionType.Exp
    Sqrt = mybir.ActivationFunctionType.Sqrt
    Copy = mybir.ActivationFunctionType.Copy
    Ident = mybir.ActivationFunctionType.Identity
    DR = mybir.MatmulPerfMode.DoubleRow

    nc = bacc.Bacc("TRN2")

    xt_d = nc.dram_tensor("xt", [D, S], fp8, kind="ExternalInput")
    xq_d = nc.dram_tensor("xq", [SQ, D], f32, kind="ExternalInput")
    wq_d = nc.dram_tensor("wqt", [D, D], fp8, kind="ExternalInput")
    wk_d = nc.dram_tensor("wkt", [D, D], fp8, kind="ExternalInput")
    wv_d = nc.dram_tensor("wvt", [D, D], fp8, kind="ExternalInput")
    wo_d = nc.dram_tensor("wot", [D, D], fp8, kind="ExternalInput")
    bq_d = nc.dram_tensor("bqt", [128, 8], f32, kind="ExternalInput")
    bk_d = nc.dram_tensor("bkt", [128, 8], f32, kind="ExternalInput")
    bv_d = nc.dram_tensor("bv", [D], f32, kind="ExternalInput")
    bo_d = nc.dram_tensor("bo", [D], f32, kind="ExternalInput")
    gamma_d = nc.dram_tensor("gamma", [D], f32, kind="ExternalInput")
    beta_d = nc.dram_tensor("beta", [D], f32, kind="ExternalInput")
    sel8_d = nc.dram_tensor("sel8", [8, 4, 128], bf16, kind="ExternalInput")
    ident_d = nc.dram_tensor("ident", [128, 128], f32, kind="ExternalInput")
    out_d = nc.dram_tensor("out", [SQ, D], f32, kind="ExternalOutput")

    def bcast_ap(handle):
        ap = handle[:]
        return bass.AP(tensor=ap.tensor, offset=ap.offset, ap=[[0, 128], ap.ap[0]])

    # which (hh, kc) score tiles go to ScalarE (exact exp) vs DVE (schraudolph)
    scalar_set = {i for i in range(32) if (i * 17) % 32 < 15}

    with tile.TileContext(nc) as tc:
        with (
            tc.tile_pool(name="const", bufs=1) as constp,
            tc.tile_pool(name="v", bufs=1) as vp,
            tc.tile_pool(name="ot", bufs=1) as otp,
            tc.tile_pool(name="xt", bufs=1) as xtp,
            tc.tile_pool(name="wo", bufs=1) as wop,
        ):
            # --- constants ---
            bq_c = constp.tile([128, 8], f32, tag="bq")
            bk_c = constp.tile([128, 8], f32, tag="bk")
            bv_c = constp.tile([128, D], f32, tag="bv")
            gamma_c = constp.tile([128, D], f32, tag="gamma")
            beta_c = constp.tile([128, D], f32, tag="beta")
            bo_c = constp.tile([128, D], f32, tag="bo")
            eps_c = constp.tile([128, 1], f32, tag="eps")
            nshift_c = constp.tile([128, 1], f32, tag="nshift")
            sel8_c = constp.tile([8, 4, 128], bf16, tag="sel8")
            if use_bq:
                nc.sync.dma_start(out=bq_c[:], in_=bq_d[:])
            if use_bk:
                nc.sync.dma_start(out=bk_c[:], in_=bk_d[:])
            if use_bv:
                nc.gpsimd.dma_start(out=bv_c[:], in_=bcast_ap(bv_d))
            if use_bo:
                nc.gpsimd.dma_start(out=bo_c[:], in_=bcast_ap(bo_d))
            if use_gamma:
                nc.gpsimd.dma_start(out=gamma_c[:], in_=bcast_ap(gamma_d))
            if use_beta:
                nc.gpsimd.dma_start(out=beta_c[:], in_=bcast_ap(beta_d))
            nc.sync.dma_start(out=sel8_c[:], in_=sel8_d[:])
            nc.vector.memset(eps_c[:], EPS)
            nc.vector.memset(nshift_c[:], -SHIFT)

            # --- persistent activations ---
            v8 = vp.tile([128, 16, H, HD + 1], fp8, tag="v")   # V + ones col (den)
            otb = otp.tile([128, 8, SQ], bf16, tag="otb")      # O^T unnormalized
            ot8 = otp.tile([128, 8, SQ], fp8, tag="ot8")       # O^T normalized
            den_a = otp.tile([8, SQ], bf16, tag="den_a")       # heads 0-7
            den_b = otp.tile([8, SQ], bf16, tag="den_b")       # heads 8-15
            xt = xtp.tile([128, 8, S], fp8, tag="xt")
            wo_r = wop.tile([128, 8, D], fp8, tag="wor")

            nc.vector.memset(v8[:, :, :, HD : HD + 1], 1.0)
            for r in range(8):
                nc.sync.dma_start(out=xt[:, r, :], in_=xt_d[r * 128 : (r + 1) * 128, :])
                nc.gpsimd.dma_start(out=wo_r[:, r, :], in_=wo_d[r * 128 : (r + 1) * 128, :])

            with (
                tc.tile_pool(name="wvr", bufs=1) as wvrp,
                tc.tile_pool(name="qkw", bufs=2) as qkwp,
                tc.tile_pool(name="qts", bufs=2) as qtsp,
                tc.tile_pool(name="kts", bufs=2) as ktsp,
                tc.tile_pool(name="st", bufs=16) as stp,
                tc.tile_pool(name="stage", bufs=4) as stagep,
                tc.tile_pool(name="rc", bufs=2) as rcp,
                tc.tile_pool(name="ps1", bufs=2, space="PSUM") as ps1,
                tc.tile_pool(name="sp", bufs=2, space="PSUM") as spp,
                tc.tile_pool(name="av", bufs=2, space="PSUM") as avp,
            ):
                # ---------- piecewise emission helpers ----------
                wv_r = wvrp.tile([128, 8, D], fp8, tag="wvr")
                for k in range(8):
                    nc.sync.dma_start(
                        out=wv_r[:, k, :], in_=wv_d[k * 128 : (k + 1) * 128, :]
                    )

                def v_chain(tc_i, dg):
                    psv = ps1.tile([128, 512], f32, tag="ps", name="psv")
                    for k in range(4):
                        nc.tensor.matmul(
                            out=psv[:],
                            lhsT=xt[:, 2 * k : 2 * k + 2, tc_i * 128 : (tc_i + 1) * 128],
                            rhs=wv_r[:, 2 * k : 2 * k + 2, dg * 512 : (dg + 1) * 512],
                            start=(k == 0),
                            stop=(k == 3),
                            perf_mode=DR,
                        )
                    dst = v8[:, tc_i, dg * 8 : (dg + 1) * 8, 0:HD]
                    if use_bv:
                        nc.vector.tensor_tensor(
                            out=dst,
                            in0=psv[:].rearrange("p (h d) -> p h d", d=HD),
                            in1=bv_c[:, dg * 512 : (dg + 1) * 512].rearrange(
                                "p (h d) -> p h d", d=HD
                            ),
                            op=ADD,
                        )
                    else:
                        nc.vector.tensor_copy(
                            out=dst, in_=psv[:].rearrange("p (h d) -> p h d", d=HD)
                        )

                pair_qt = {}

                def proj_piece(m, j):
                    """j=0: wq DMA + Q chain tg0; j=1: Q tg1; j=2: wk DMA + K tg0;
                    j=3..5: K tg1..3."""
                    st = pair_qt.setdefault(m, {})
                    if j == 0:
                        wq_m = qkwp.tile([128, 8, 128], fp8, tag="qkw", name="wq_m")
                        nc.sync.dma_start(
                            out=wq_m[:],
                            in_=wq_d[:, m * 128 : (m + 1) * 128].rearrange(
                                "(k p) c -> p k c", p=128
                            ),
                        )
                        st["wq"] = wq_m
                        st["qt"] = qtsp.tile([128, SQ], bf16, tag="qts", name="qt_m")
                    if j == 2:
                        wk_m = qkwp.tile([128, 8, 128], fp8, tag="qkw", name="wk_m")
                        nc.sync.dma_start(
                            out=wk_m[:],
                            in_=wk_d[:, m * 128 : (m + 1) * 128].rearrange(
                                "(k p) c -> p k c", p=128
                            ),
                        )
                        st["wk"] = wk_m
                        st["kt"] = ktsp.tile([128, S], bf16, tag="kts", name="kt_m")
                    if j < 2:
                        w, dstt, tg, bias_c, use_b = st["wq"], st["qt"], j, bq_c, use_bq
                    else:
                        w, dstt, tg, bias_c, use_b = st["wk"], st["kt"], j - 2, bk_c, use_bk
                    ps = ps1.tile([128, 512], f32, tag="ps", name="psqk")
                    for k in range(4):
                        nc.tensor.matmul(
                            out=ps[:],
                            lhsT=w[:, 2 * k : 2 * k + 2, :],
                            rhs=xt[:, 2 * k : 2 * k + 2, tg * 512 : (tg + 1) * 512],
                            start=(k == 0),
                            stop=(k == 3),
                            perf_mode=DR,
                        )
                    if use_b:
                        nc.scalar.activation(
                            out=dstt[:, tg * 512 : (tg + 1) * 512],
                            in_=ps[:],
                            func=Ident,
                            bias=bias_c[:, m : m + 1],
                        )
                    else:
                        nc.scalar.copy(
                            out=dstt[:, tg * 512 : (tg + 1) * 512], in_=ps[:]
                        )

                def qk_exp_kc(m, kc, qt_m, kt_m, st_pair):
                    sps = [
                        spp.tile([128, 1024], f32, tag="sp", name="sp") for _ in range(2)
                    ]
                    for qh in range(2):
                        for hh in range(2):
                            p0 = hh * 64
                            nc.tensor.matmul(
                                out=sps[hh][:, qh * 512 : (qh + 1) * 512],
                                lhsT=kt_m[p0 : p0 + 64, kc * 128 : (kc + 1) * 128],
                                rhs=qt_m[p0 : p0 + 64, qh * 512 : (qh + 1) * 512],
                                start=True,
                                stop=True,
                            )
                    for hh in range(2):
                        dst = st_pair[hh][kc // 4][:, kc % 4, :]
                        if (2 * kc + hh) in scalar_set:
                            nc.scalar.activation(
                                out=dst,
                                in_=sps[hh][:],
                                func=Exp,
                                scale=0.125,
                                bias=nshift_c[:, 0:1],
                            )
                        else:
                            nc.vector.tensor_scalar(
                                out=dst.bitcast(u8),
                                in0=sps[hh][:],
                                scalar1=float(A_DVE),
                                scalar2=float(B_DVE),
                                op0=MULT,
                                op1=ADD,
                            )

                av_stg = {}

                def av_piece(m, piece, st_pair):
                    """piece = 2*hh + qh; 8 DR accums + evac; DMAs at hh ends."""
                    den_t = den_a if m < 4 else den_b
                    hh, qh = piece // 2, piece % 2
                    h = 2 * m + hh
                    st_tiles = st_pair[hh]
                    if qh == 0:
                        av_stg[(m, hh)] = stagep.tile(
                            [65, 2, 512], bf16, tag="stg", name="stg"
                        )
                    stg = av_stg[(m, hh)]
                    av = avp.tile([128, 512], f32, tag="av", name="av")
                    for c in range(8):
                        u, jj = c // 2, c % 2
                        nc.tensor.matmul(
                            out=av[0:65, :],
                            lhsT=v8[:, 4 * u + 2 * jj : 4 * u + 2 * jj + 2, h, :],
                            rhs=st_tiles[u][:, 2 * jj : 2 * jj + 2, qh * 512 : (qh + 1) * 512],
                            start=(c == 0),
                            stop=(c == 7),
                            perf_mode=DR,
                        )
                    if hh == 0:
                        nc.vector.tensor_copy(
                            out=otb[0:64, m, qh * 512 : (qh + 1) * 512],
                            in_=av[0:64, :],
                        )
                        nc.scalar.copy(out=stg[64:65, qh, :], in_=av[64:65, :])
                    else:
                        nc.scalar.copy(out=stg[0:65, qh, :], in_=av[0:65, :])
                    if qh == 1:
                        if hh == 0:
                            nc.sync.dma_start(
                                out=den_t[2 * (m % 4) : 2 * (m % 4) + 1, :],
                                in_=stg[64:65, :, :],
                            )
                        else:
                            nc.sync.dma_start(
                                out=otb[64:128, m, :], in_=stg[0:64, :, :]
                            )
                            nc.sync.dma_start(
                                out=den_t[2 * (m % 4) + 1 : 2 * (m % 4) + 2, :],
                                in_=stg[64:65, :, :],
                            )

                def norm_batch(b_i):
                    den_t = den_a if b_i == 0 else den_b
                    rc_f = rcp.tile([8, SQ], f32, tag="rcf", name="rc_f")
                    rc_b = rcp.tile([8, SQ], bf16, tag="rcb", name="rc_b")
                    nc.vector.reciprocal(out=rc_f[:], in_=den_t[:])
                    nc.vector.tensor_copy(out=rc_b[:], in_=rc_f[:])
                    for mm in range(4):
                        m = 4 * b_i + mm
                        for qh in range(2):
                            bc = ps1.tile([128, 512], f32, tag="ps", name="bc")
                            nc.tensor.matmul(
                                out=bc[:],
                                lhsT=sel8_c[:, mm, :],
                                rhs=rc_b[:, qh * 512 : (qh + 1) * 512],
                                start=True,
                                stop=True,
                            )
                            nc.vector.tensor_tensor(
                                out=ot8[:, m, qh * 512 : (qh + 1) * 512],
                                in0=otb[:, m, qh * 512 : (qh + 1) * 512],
                                in1=bc[:],
                                op=MULT,
                            )

                # ---------- interleaved pipeline ----------
                pair_st = {}
                for jj in range(6):
                    proj_piece(0, jj)
                for m in range(8):
                    qt_m = pair_qt[m]["qt"]
                    kt_m = pair_qt[m]["kt"]
                    st_pair = [
                        [stp.tile([128, 4, SQ], fp8e5, tag="st", name="st") for _ in range(4)]
                        for _ in range(2)
                    ]
                    pair_st[m] = st_pair
                    for kc in range(16):
                        qk_exp_kc(m, kc, qt_m, kt_m, st_pair)
                        if m == 0:
                            # fold the V projection into pair 0's loop
                            v_chain(kc, 0)
                            v_chain(kc, 1)
                        elif kc % 4 == 3:
                            av_piece(m - 1, kc // 4, pair_st[m - 1])
                        if m < 7 and kc % 2 == 0 and kc < 12:
                            proj_piece(m + 1, kc // 2)
                    if m >= 2:
                        del pair_st[m - 2]
                    if m == 4:
                        norm_batch(0)
                av_piece(7, 0, pair_st[7])
                av_piece(7, 1, pair_st[7])
                av_piece(7, 2, pair_st[7])
                av_piece(7, 3, pair_st[7])
                norm_batch(1)

            # ========== phase 3: O projection + residual + LN ==========
            with (
                tc.tile_pool(name="xqp", bufs=8) as xqp,
                tc.tile_pool(name="id", bufs=1) as idp,
                tc.tile_pool(name="yo", bufs=3) as yop,
                tc.tile_pool(name="stats", bufs=4) as statp,
                tc.tile_pool(name="ps3", bufs=3, space="PSUM") as ps3,
            ):
                ident = idp.tile([128, 128], f32r, tag="ident")
                nc.gpsimd.dma_start(out=ident[:], in_=ident_d[:])
                xq_tiles = []
                for t in range(8):
                    xq_t = xqp.tile([128, D], f32r, tag="xq", name="xq_t")
                    nc.gpsimd.dma_start(out=xq_t[:], in_=xq_d[t * 128 : (t + 1) * 128, :])
                    xq_tiles.append(xq_t)
                if use_bo:
                    for t in range(8):
                        nc.gpsimd.tensor_tensor(
                            out=xq_tiles[t][:], in0=xq_tiles[t][:], in1=bo_c[:], op=ADD
                        )
                for t in range(8):
                    ps = ps3.tile([128, D], f32, tag="ps3", name="ps3")
                    for g in range(2):
                        for k in range(4):
                            nc.tensor.matmul(
                                out=ps[:, g * 512 : (g + 1) * 512],
                                lhsT=ot8[:, 2 * k : 2 * k + 2, t * 128 : (t + 1) * 128],
                                rhs=wo_r[:, 2 * k : 2 * k + 2, g * 512 : (g + 1) * 512],
                                start=(k == 0),
                                stop=False,
                                perf_mode=DR,
                                skip_group_check=True,
                            )
                        # residual: accumulate X via fp32r identity matmul
                        nc.tensor.matmul(
                            out=ps[:, g * 512 : (g + 1) * 512],
                            lhsT=ident[:],
                            rhs=xq_tiles[t][:, g * 512 : (g + 1) * 512],
                            start=False,
                            stop=True,
                            skip_group_check=True,
                        )
                    stats = statp.tile([128, 2, 6], f32, tag="stats")
                    mv = statp.tile([128, 2], f32, tag="mv")
                    mr = statp.tile([128, 1], f32, tag="mr")
                    nc.vector.bn_stats(out=stats[:, 0, :], in_=ps[:, 0:512])
                    nc.vector.bn_stats(out=stats[:, 1, :], in_=ps[:, 512:1024])
                    nc.vector.bn_aggr(out=mv[:], in_=stats[:])
                    nc.scalar.activation(
                        out=mv[:, 1:2], in_=mv[:, 1:2], func=Sqrt, bias=eps_c[:, 0:1]
                    )
                    nc.vector.reciprocal(out=mv[:, 1:2], in_=mv[:, 1:2])
                    nc.vector.tensor_scalar(
                        out=mr[:],
                        in0=mv[:, 0:1],
                        scalar1=mv[:, 1:2],
                        scalar2=-1.0,
                        op0=MULT,
                        op1=MULT,
                    )
                    yo = yop.tile([128, D], f32, tag="yo")
                    nc.scalar.activation(
                        out=yo[:], in_=ps[:], func=Ident,
                        scale=mv[:, 1:2], bias=mr[:, 0:1],
                    )
                    if use_gamma:
                        nc.vector.tensor_tensor(out=yo[:], in0=yo[:], in1=gamma_c[:], op=MULT)
                    if use_beta:
                        nc.gpsimd.tensor_tensor(out=yo[:], in0=yo[:], in1=beta_c[:], op=ADD)
                    nc.sync.dma_start(out=out_d[t * 128 : (t + 1) * 128, :], in_=yo[:])

    nc.compile()
    return nc


def _get_nc(flags):
    key = ("nc", flags)
    if key not in _CACHE:
        _CACHE[key] = _build(flags)
    return _CACHE[key]


def kernel(X, Wq, bq, Wk, bk, Wv, bv, Wo, bo, gamma, beta):
    if os.environ.get("BASS_TRACE"):
        _install_ntff_hook()
    import ml_dtypes

    from concourse.bass_utils import run_bass_kernel_spmd

    f8 = ml_dtypes.float8_e4m3
    bfdt = ml_dtypes.bfloat16
    f32 = np.float32
    X = np.ascontiguousarray(np.asarray(X, dtype=f32))
    bq_ = np.asarray(bq, f32)
    bk_ = np.asarray(bk, f32)
    bv_ = np.ascontiguousarray(np.asarray(bv, f32))
    bo_ = np.ascontiguousarray(np.asarray(bo, f32))
    gamma_ = np.ascontiguousarray(np.asarray(gamma, f32))
    beta_ = np.ascontiguousarray(np.asarray(beta, f32))
    flags = (
        bool(np.any(bq_)), bool(np.any(bk_)), bool(np.any(bv_)), bool(np.any(bo_)),
        bool(np.any(gamma_ != 1.0)), bool(np.any(beta_)),
    )

    wqt = np.ascontiguousarray(np.asarray(Wq, f32).T.astype(f8))
    wkt = np.ascontiguousarray(np.asarray(Wk, f32).T.astype(f8))
    wvt = np.ascontiguousarray(np.asarray(Wv, f32).T.astype(f8))
    wot = np.ascontiguousarray(np.asarray(Wo, f32).T.astype(f8))
    bqt = np.ascontiguousarray(bq_.reshape(8, 128).T)
    bkt = np.ascontiguousarray(bk_.reshape(8, 128).T)
    sel8 = np.zeros((8, 4, 128), f32)
    for mm in range(4):
        for p in range(128):
            sel8[2 * mm + p // 64, mm, p] = 1.0
    sel8 = sel8.astype(bfdt)
    ident = np.eye(128, dtype=f32)

    in_maps = []
    for c in range(NCORES):
        b, half = c // 2, c % 2
        Xb = X[b]
        q_rows = Xb[half * SQ : (half + 1) * SQ]
        o_rows = Xb[(1 - half) * SQ : (2 - half) * SQ]
        # queries-first token order (key order is permutation-invariant)
        xt = np.ascontiguousarray(np.concatenate([q_rows, o_rows], axis=0).T.astype(f8))
        in_maps.append(
            {
                "xt": xt,
                "xq": np.ascontiguousarray(q_rows),
                "wqt": wqt,
                "wkt": wkt,
                "wvt": wvt,
                "wot": wot,
                "bqt": bqt,
                "bkt": bkt,
                "bv": bv_,
                "bo": bo_,
                "gamma": gamma_,
                "beta": beta_,
                "sel8": sel8,
                "ident": ident,
            }
        )

    nc = _get_nc(flags)
    res = run_bass_kernel_spmd(nc, in_maps, core_ids=list(range(NCORES)))
    if res.exec_time_ns is not None:
        print(f"HW exec time: {res.exec_time_ns} ns")

    out = np.empty((B, S, D), np.float32)
    for c in range(NCORES):
        b, half = c // 2, c % 2
        out[b, half * SQ : (half + 1) * SQ] = res.results[c]["out"]
    return out


# revision 8
# speedup vs baseline: 1.3146x; 1.0155x over previous
"""Trainium2 Bass kernel for nn_Attention (B=4, S=2048, D=1024, H=16) on 8 NeuronCores.

Sharding: data-parallel over (batch, sequence-half) -> 8 shards, one per core.
Each core computes attention for 1024 query tokens of one batch element.

v2: fp8 rework of the bf16 baseline.
 - All projections (Q/K/V/O) run as fp8e4m3 DoubleRow matmuls: contraction of
   2x128 partitions per instruction at the same issue rate as bf16, i.e. 2x
   the throughput.
 - QK^T stays bf16 but alternates the 64-partition stationary base between
   head 0 (partitions 0:64) and head 1 (64:128) of each pair, so the PE
   loads one head's keys while streaming the other's queries (~2x issue rate).
 - Attention weights A = exp(s/8 - 4?) are stored in fp8e5m2 (wide dynamic
   range; raw scores reach 73 > e4m3 budget). The exp stream - the biggest
   elementwise cost - is split between ScalarE (exact exp, fp8 output) and
   DVE (Schraudolph: affine in log2 domain + saturating round to uint8 IS the
   fp8e5m2 bit pattern). A@V contracts A (e5m2) against V (e4m3) in DoubleRow
   mode with a ones-column folding the softmax denominator into the matmul.
 - Denominators collect into [8,S] tiles; batched reciprocal + per-pair
   broadcast matmul (sel8) normalizes O^T, written fp8 for the O projection.
 - Residual + LayerNorm in fp32; the (x-mu)*rstd affine runs on ScalarE with
   per-partition scale/bias APs.
Zero biases / identity gamma,beta (checked on host) skip their instructions.
"""

import os
import sys

sys.path.insert(0, "/opt/trn_rl_repo")

import numpy as np

B, S, D, H = 4, 2048, 1024, 16
HD = D // H  # 64
SQ = S // 2  # queries per core
NCORES = 8
EPS = 1e-12

SHIFT = 2.5
LOG2E = 1.4426950408889634
A_DVE = LOG2E * 0.125 * 4               # schraudolph slope per raw score (e5m2)
B_DVE = 60.0 - 0.225 - 4 * SHIFT * LOG2E  # fitted offset (c=-0.225)

_CACHE = {}


def _install_ntff_hook():
    """Register the axon NTFF profile hook that bass_utils looks up via
    antenv.axon_hooks (absent from the image's antenv stub)."""
    import contextlib
    import ctypes
    import types

    so_path = "/opt/axon/libaxon_pjrt.so"
    if "antenv.axon_hooks" in sys.modules:
        return
    try:
        lib = ctypes.CDLL(so_path)
    except OSError:
        return
    if not hasattr(lib, "axon_start_nrt_profile"):
        return
    lib.axon_start_nrt_profile.argtypes = [ctypes.POINTER(ctypes.c_int64), ctypes.c_size_t]
    lib.axon_start_nrt_profile.restype = ctypes.c_int64
    lib.axon_stop_nrt_profile.argtypes = [ctypes.c_char_p]
    lib.axon_stop_nrt_profile.restype = ctypes.c_int64

    @contextlib.contextmanager
    def _hook(output_dir, device_ids):
        import jax

        jax.devices()
        if device_ids:
            ids = (ctypes.c_int64 * len(device_ids))(*device_ids)
            rc = lib.axon_start_nrt_profile(ids, len(device_ids))
        else:
            rc = lib.axon_start_nrt_profile(None, 0)
        if rc != 0:
            raise RuntimeError(f"axon_start_nrt_profile rc={rc}")
        try:
            yield
        finally:
            n = lib.axon_stop_nrt_profile(str(output_dir).encode())
            if n < 0:
                raise RuntimeError(f"axon_stop_nrt_profile rc={n}")

    m = types.ModuleType("antenv.axon_hooks")
    m.get_axon_ntff_profile_hook = lambda: _hook
    m.set_axon_ntff_profile_hook = lambda h: None
    sys.modules["antenv.axon_hooks"] = m


def _build(flags):
    use_bq, use_bk, use_bv, use_bo, use_gamma, use_beta = flags

    import concourse.bass as bass
    import concourse.tile as tile
    from concourse import bacc, mybir

    f32 = mybir.dt.float32
    bf16 = mybir.dt.bfloat16
    fp8 = mybir.dt.float8e4
    fp8e5 = mybir.dt.float8e5
    f32r = mybir.dt.float32r
    u8 = mybir.dt.uint8
    ADD = mybir.AluOpType.add
    MULT = mybir.AluOpType.mult
    SUB = mybir.AluOpType.subtract
    Exp = mybir.ActivationFunctionType.Exp
    Sqrt = mybir.ActivationFunctionType.Sqrt
    Copy = mybir.ActivationFunctionType.Copy
    Ident = mybir.ActivationFunctionType.Identity
    DR = mybir.MatmulPerfMode.DoubleRow

    nc = bacc.Bacc("TRN2")

    xt_d = nc.dram_tensor("xt", [D, S], fp8, kind="ExternalInput")
    xq_d = nc.dram_tensor("xq", [SQ, D], f32, kind="ExternalInput")
    wq_d = nc.dram_tensor("wqt", [D, D], fp8, kind="ExternalInput")
    wk_d = nc.dram_tensor("wkt", [D, D], fp8, kind="ExternalInput")
    wv_d = nc.dram_tensor("wvt", [D, D], fp8, kind="ExternalInput")
    wo_d = nc.dram_tensor("wot", [D, D], fp8, kind="ExternalInput")
    bq_d = nc.dram_tensor("bqt", [128, 8], f32, kind="ExternalInput")
    bk_d = nc.dram_tensor("bkt", [128, 8], f32, kind="ExternalInput")
    bv_d = nc.dram_tensor("bv", [D], f32, kind="ExternalInput")
    bo_d = nc.dram_tensor("bo", [D], f32, kind="ExternalInput")
    gamma_d = nc.dram_tensor("gamma", [D], f32, kind="ExternalInput")
    beta_d = nc.dram_tensor("beta", [D], f32, kind="ExternalInput")
    sel8_d = nc.dram_tensor("sel8", [8, 4, 128], bf16, kind="ExternalInput")
    ident_d = nc.dram_tensor("ident", [128, 128], f32, kind="ExternalInput")
    out_d = nc.dram_tensor("out", [SQ, D], f32, kind="ExternalOutput")

    def bcast_ap(handle):
        ap = handle[:]
        return bass.AP(tensor=ap.tensor, offset=ap.offset, ap=[[0, 128], ap.ap[0]])

    # which (hh, kc) score tiles go to ScalarE (exact exp) vs DVE (schraudolph)
    scalar_set = {i for i in range(32) if (i * 17) % 32 < 15}

    with tile.TileContext(nc) as tc:
        with (
            tc.tile_pool(name="const", bufs=1) as constp,
            tc.tile_pool(name="v", bufs=1) as vp,
            tc.tile_pool(name="ot", bufs=1) as otp,
            tc.tile_pool(name="xt", bufs=1) as xtp,
            tc.tile_pool(name="wo", bufs=1) as wop,
        ):
            # --- constants ---
            bq_c = constp.tile([128, 8], f32, tag="bq")
            bk_c = constp.tile([128, 8], f32, tag="bk")
            bv_c = constp.tile([128, D], f32, tag="bv")
            gamma_c = constp.tile([128, D], f32, tag="gamma")
            beta_c = constp.tile([128, D], f32, tag="beta")
            bo_c = constp.tile([128, D], f32, tag="bo")
            eps_c = constp.tile([128, 1], f32, tag="eps")
            nshift_c = constp.tile([128, 1], f32, tag="nshift")
            sel8_c = constp.tile([8, 4, 128], bf16, tag="sel8")
            if use_bq:
                nc.sync.dma_start(out=bq_c[:], in_=bq_d[:])
            if use_bk:
                nc.sync.dma_start(out=bk_c[:], in_=bk_d[:])
            if use_bv:
                nc.gpsimd.dma_start(out=bv_c[:], in_=bcast_ap(bv_d))
            if use_bo:
                nc.gpsimd.dma_start(out=bo_c[:], in_=bcast_ap(bo_d))
            if use_gamma:
                nc.gpsimd.dma_start(out=gamma_c[:], in_=bcast_ap(gamma_d))
            if use_beta:
                nc.gpsimd.dma_start(out=beta_c[:], in_=bcast_ap(beta_d))
            nc.sync.dma_start(out=sel8_c[:], in_=sel8_d[:])
            nc.vector.memset(eps_c[:], EPS)
            nc.vector.memset(nshift_c[:], -SHIFT)

            # --- persistent activations ---
            v8 = vp.tile([128, 16, H, HD + 1], fp8, tag="v")   # V + ones col (den)
            otb = otp.tile([128, 8, SQ], bf16, tag="otb")      # O^T unnormalized
            ot8 = otp.tile([128, 8, SQ], fp8, tag="ot8")       # O^T normalized
            den_a = otp.tile([8, SQ], bf16, tag="den_a")       # heads 0-7
            den_b = otp.tile([8, SQ], bf16, tag="den_b")       # heads 8-13
            den_c = otp.tile([2, SQ], bf16, tag="den_c")       # heads 14-15
            xt = xtp.tile([128, 8, S], fp8, tag="xt")
            wo_r = wop.tile([128, 8, D], fp8, tag="wor")

            nc.vector.memset(v8[:, :, :, HD : HD + 1], 1.0)
            for r in range(8):
                nc.sync.dma_start(out=xt[:, r, :], in_=xt_d[r * 128 : (r + 1) * 128, :])
                nc.gpsimd.dma_start(out=wo_r[:, r, :], in_=wo_d[r * 128 : (r + 1) * 128, :])

            with (
                tc.tile_pool(name="wvr", bufs=1) as wvrp,
                tc.tile_pool(name="qkw", bufs=2) as qkwp,
                tc.tile_pool(name="qts", bufs=2) as qtsp,
                tc.tile_pool(name="kts", bufs=2) as ktsp,
                tc.tile_pool(name="st", bufs=16) as stp,
                tc.tile_pool(name="stage", bufs=4) as stagep,
                tc.tile_pool(name="rc", bufs=1) as rcp,
                tc.tile_pool(name="ps1", bufs=2, space="PSUM") as ps1,
                tc.tile_pool(name="sp", bufs=2, space="PSUM") as spp,
                tc.tile_pool(name="av", bufs=2, space="PSUM") as avp,
            ):
                # ---------- piecewise emission helpers ----------
                wv_r = wvrp.tile([128, 8, D], fp8, tag="wvr")
                for k in range(8):
                    nc.sync.dma_start(
                        out=wv_r[:, k, :], in_=wv_d[k * 128 : (k + 1) * 128, :]
                    )

                def v_chain(tc_i, dg):
                    psv = ps1.tile([128, 512], f32, tag="ps", name="psv")
                    for k in range(4):
                        nc.tensor.matmul(
                            out=psv[:],
                            lhsT=xt[:, 2 * k : 2 * k + 2, tc_i * 128 : (tc_i + 1) * 128],
                            rhs=wv_r[:, 2 * k : 2 * k + 2, dg * 512 : (dg + 1) * 512],
                            start=(k == 0),
                            stop=(k == 3),
                            perf_mode=DR,
                        )
                    dst = v8[:, tc_i, dg * 8 : (dg + 1) * 8, 0:HD]
                    if use_bv:
                        nc.vector.tensor_tensor(
                            out=dst,
                            in0=psv[:].rearrange("p (h d) -> p h d", d=HD),
                            in1=bv_c[:, dg * 512 : (dg + 1) * 512].rearrange(
                                "p (h d) -> p h d", d=HD
                            ),
                            op=ADD,
                        )
                    else:
                        nc.vector.tensor_copy(
                            out=dst, in_=psv[:].rearrange("p (h d) -> p h d", d=HD)
                        )

                pair_qt = {}

                def proj_piece(m, j):
                    """j=0: wq DMA + Q chain tg0; j=1: Q tg1; j=2: wk DMA + K tg0;
                    j=3..5: K tg1..3."""
                    st = pair_qt.setdefault(m, {})
                    if j == 0:
                        wq_m = qkwp.tile([128, 8, 128], fp8, tag="qkw", name="wq_m")
                        nc.sync.dma_start(
                            out=wq_m[:],
                            in_=wq_d[:, m * 128 : (m + 1) * 128].rearrange(
                                "(k p) c -> p k c", p=128
                            ),
                        )
                        st["wq"] = wq_m
                        st["qt"] = qtsp.tile([128, SQ], bf16, tag="qts", name="qt_m")
                    if j == 2:
                        wk_m = qkwp.tile([128, 8, 128], fp8, tag="qkw", name="wk_m")
                        nc.sync.dma_start(
                            out=wk_m[:],
                            in_=wk_d[:, m * 128 : (m + 1) * 128].rearrange(
                                "(k p) c -> p k c", p=128
                            ),
                        )
                        st["wk"] = wk_m
                        st["kt"] = ktsp.tile([128, S], bf16, tag="kts", name="kt_m")
                    if j < 2:
                        w, dstt, tg, bias_c, use_b = st["wq"], st["qt"], j, bq_c, use_bq
                    else:
                        w, dstt, tg, bias_c, use_b = st["wk"], st["kt"], j - 2, bk_c, use_bk
                    ps = ps1.tile([128, 512], f32, tag="ps", name="psqk")
                    for k in range(4):
                        nc.tensor.matmul(
                            out=ps[:],
                            lhsT=w[:, 2 * k : 2 * k + 2, :],
                            rhs=xt[:, 2 * k : 2 * k + 2, tg * 512 : (tg + 1) * 512],
                            start=(k == 0),
                            stop=(k == 3),
                            perf_mode=DR,
                        )
                    if use_b:
                        nc.scalar.activation(
                            out=dstt[:, tg * 512 : (tg + 1) * 512],
                            in_=ps[:],
                            func=Ident,
                            bias=bias_c[:, m : m + 1],
                        )
                    else:
                        nc.scalar.copy(
                            out=dstt[:, tg * 512 : (tg + 1) * 512], in_=ps[:]
                        )

                def qk_exp_kc(m, kc, qt_m, kt_m, st_pair):
                    sps = [
                        spp.tile([128, 1024], f32, tag="sp", name="sp") for _ in range(2)
                    ]
                    for qh in range(2):
                        for hh in range(2):
                            p0 = hh * 64
                            nc.tensor.matmul(
                                out=sps[hh][:, qh * 512 : (qh + 1) * 512],
                                lhsT=kt_m[p0 : p0 + 64, kc * 128 : (kc + 1) * 128],
                                rhs=qt_m[p0 : p0 + 64, qh * 512 : (qh + 1) * 512],
                                start=True,
                                stop=True,
                            )
                    for hh in range(2):
                        dst = st_pair[hh][kc // 4][:, kc % 4, :]
                        if (2 * kc + hh) in scalar_set:
                            nc.scalar.activation(
                                out=dst,
                                in_=sps[hh][:],
                                func=Exp,
                                scale=0.125,
                                bias=nshift_c[:, 0:1],
                            )
                        else:
                            nc.vector.tensor_scalar(
                                out=dst.bitcast(u8),
                                in0=sps[hh][:],
                                scalar1=float(A_DVE),
                                scalar2=float(B_DVE),
                                op0=MULT,
                                op1=ADD,
                            )

                av_stg = {}

                def av_piece(m, piece, st_pair):
                    """piece = 2*hh + qh; 8 DR accums + evac; DMAs at hh ends."""
                    den_t = den_a if m < 4 else (den_b if m < 7 else den_c)
                    den_r = 2 * (m % 4) if m < 7 else 0
                    hh, qh = piece // 2, piece % 2
                    h = 2 * m + hh
                    st_tiles = st_pair[hh]
                    if qh == 0:
                        av_stg[(m, hh)] = stagep.tile(
                            [65, 2, 512], bf16, tag="stg", name="stg"
                        )
                    stg = av_stg[(m, hh)]
                    av = ps1.tile([128, 512], f32, tag="ps", name="av")
                    for c in range(8):
                        u, jj = c // 2, c % 2
                        nc.tensor.matmul(
                            out=av[0:65, :],
                            lhsT=v8[:, 4 * u + 2 * jj : 4 * u + 2 * jj + 2, h, :],
                            rhs=st_tiles[u][:, 2 * jj : 2 * jj + 2, qh * 512 : (qh + 1) * 512],
                            start=(c == 0),
                            stop=(c == 7),
                            perf_mode=DR,
                        )
                    if hh == 0:
                        nc.vector.tensor_copy(
                            out=otb[0:64, m, qh * 512 : (qh + 1) * 512],
                            in_=av[0:64, :],
                        )
                        nc.scalar.copy(out=stg[64:65, qh, :], in_=av[64:65, :])
                    else:
                        nc.scalar.copy(out=stg[0:65, qh, :], in_=av[0:65, :])
                    if qh == 1:
                        if hh == 0:
                            nc.sync.dma_start(
                                out=den_t[den_r : den_r + 1, :],
                                in_=stg[64:65, :, :],
                            )
                        else:
                            nc.sync.dma_start(
                                out=otb[64:128, m, :], in_=stg[0:64, :, :]
                            )
                            nc.sync.dma_start(
                                out=den_t[den_r + 1 : den_r + 2, :],
                                in_=stg[64:65, :, :],
                            )

                def norm_batch(b_i):
                    # b_i 0: pairs 0-3; 1: pairs 4-6; 2: pair 7
                    den_t = (den_a, den_b, den_c)[b_i]
                    nrows = (8, 6, 2)[b_i]
                    pairs = ((0, 1, 2, 3), (4, 5, 6), (7,))[b_i]
                    rc_f = rcp.tile([8, SQ], f32, tag="rcf", name="rc_f")
                    rc_b = rcp.tile([8, SQ], bf16, tag="rcb", name="rc_b")
                    nc.vector.reciprocal(out=rc_f[0:nrows, :], in_=den_t[0:nrows, :])
                    nc.vector.tensor_copy(out=rc_b[0:nrows, :], in_=rc_f[0:nrows, :])
                    for mi, m in enumerate(pairs):
                        mm = 0 if b_i == 2 else m % 4
                        for qh in range(2):
                            bc = ps1.tile([128, 512], f32, tag="ps", name="bc")
                            nc.tensor.matmul(
                                out=bc[:],
                                lhsT=sel8_c[:, mm, :],
                                rhs=rc_b[:, qh * 512 : (qh + 1) * 512],
                                start=True,
                                stop=True,
                            )
                            nc.vector.tensor_tensor(
                                out=ot8[:, m, qh * 512 : (qh + 1) * 512],
                                in0=otb[:, m, qh * 512 : (qh + 1) * 512],
                                in1=bc[:],
                                op=MULT,
                            )

                # ---------- interleaved pipeline ----------
                pair_st = {}
                vq = [(tc_i, dg) for tc_i in range(16) for dg in range(2)]
                for jj in range(6):
                    proj_piece(0, jj)
                for m in range(8):
                    qt_m = pair_qt[m]["qt"]
                    kt_m = pair_qt[m]["kt"]
                    st_pair = [
                        [stp.tile([128, 4, SQ], fp8e5, tag="st", name="st") for _ in range(4)]
                        for _ in range(2)
                    ]
                    pair_st[m] = st_pair
                    for kc in range(16):
                        qk_exp_kc(m, kc, qt_m, kt_m, st_pair)
                        if m == 0:
                            # fold the V projection into pair 0's loop
                            if vq:
                                v_chain(*vq.pop(0))
                            if vq:
                                v_chain(*vq.pop(0))
                        # pair 1 must wait for the tail of the V projection
                        if m >= 1 and kc % 4 == (3 if m == 1 else 1):
                            av_piece(m - 1, kc // 4, pair_st[m - 1])
                        if m < 7 and kc % 2 == 0 and kc < 12:
                            proj_piece(m + 1, kc // 2)
                    if m >= 2:
                        del pair_st[m - 2]
                    if m == 4:
                        norm_batch(0)
                    if m == 7:
                        norm_batch(1)
                av_piece(7, 0, pair_st[7])
                av_piece(7, 1, pair_st[7])
                av_piece(7, 2, pair_st[7])
                av_piece(7, 3, pair_st[7])
                norm_batch(2)

            # ========== phase 3: O projection + residual + LN ==========
            with (
                tc.tile_pool(name="xqp", bufs=8) as xqp,
                tc.tile_pool(name="id", bufs=1) as idp,
                tc.tile_pool(name="yo", bufs=3) as yop,
                tc.tile_pool(name="stats", bufs=4) as statp,
                tc.tile_pool(name="ps3", bufs=3, space="PSUM") as ps3,
            ):
                ident = idp.tile([128, 128], f32r, tag="ident")
                nc.gpsimd.dma_start(out=ident[:], in_=ident_d[:])
                xq_tiles = []
                for t in range(8):
                    xq_t = xqp.tile([128, D], f32r, tag="xq", name="xq_t")
                    nc.gpsimd.dma_start(out=xq_t[:], in_=xq_d[t * 128 : (t + 1) * 128, :])
                    xq_tiles.append(xq_t)
                if use_bo:
                    for t in range(8):
                        nc.gpsimd.tensor_tensor(
                            out=xq_tiles[t][:], in0=xq_tiles[t][:], in1=bo_c[:], op=ADD
                        )
                for t in range(8):
                    ps = ps3.tile([128, D], f32, tag="ps3", name="ps3")
                    for g in range(2):
                        for k in range(4):
                            nc.tensor.matmul(
                                out=ps[:, g * 512 : (g + 1) * 512],
                                lhsT=ot8[:, 2 * k : 2 * k + 2, t * 128 : (t + 1) * 128],
                                rhs=wo_r[:, 2 * k : 2 * k + 2, g * 512 : (g + 1) * 512],
                                start=(k == 0),
                                stop=False,
                                perf_mode=DR,
                                skip_group_check=True,
                            )
                        # residual: accumulate X via fp32r identity matmul
                        nc.tensor.matmul(
                            out=ps[:, g * 512 : (g + 1) * 512],
                            lhsT=ident[:],
                            rhs=xq_tiles[t][:, g * 512 : (g + 1) * 512],
                            start=False,
                            stop=True,
                            skip_group_check=True,
                        )
                    stats = statp.tile([128, 2, 6], f32, tag="stats")
                    mv = statp.tile([128, 2], f32, tag="mv")
                    mr = statp.tile([128, 1], f32, tag="mr")
                    nc.vector.bn_stats(out=stats[:, 0, :], in_=ps[:, 0:512])
                    nc.vector.bn_stats(out=stats[:, 1, :], in_=ps[:, 512:1024])
                    nc.vector.bn_aggr(out=mv[:], in_=stats[:])
                    nc.scalar.activation(
                        out=mv[:, 1:2], in_=mv[:, 1:2], func=Sqrt, bias=eps_c[:, 0:1]
                    )
                    nc.vector.reciprocal(out=mv[:, 1:2], in_=mv[:, 1:2])
                    nc.vector.tensor_scalar(
                        out=mr[:],
                        in0=mv[:, 0:1],
                        scalar1=mv[:, 1:2],
                        scalar2=-1.0,
                        op0=MULT,
                        op1=MULT,
                    )
                    yo = yop.tile([128, D], f32, tag="yo")
                    nc.scalar.activation(
                        out=yo[:], in_=ps[:], func=Ident,
                        scale=mv[:, 1:2], bias=mr[:, 0:1],
                    )
                    if use_gamma:
                        nc.vector.tensor_tensor(out=yo[:], in0=yo[:], in1=gamma_c[:], op=MULT)
                    if use_beta:
                        nc.gpsimd.tensor_tensor(out=yo[:], in0=yo[:], in1=beta_c[:], op=ADD)
                    nc.sync.dma_start(out=out_d[t * 128 : (t + 1) * 128, :], in_=yo[:])

    nc.compile()
    return nc


def _get_nc(flags):
    key = ("nc", flags)
    if key not in _CACHE:
        _CACHE[key] = _build(flags)
    return _CACHE[key]


def kernel(X, Wq, bq, Wk, bk, Wv, bv, Wo, bo, gamma, beta):
    if os.environ.get("BASS_TRACE"):
        _install_ntff_hook()
    import ml_dtypes

    from concourse.bass_utils import run_bass_kernel_spmd

    f8 = ml_dtypes.float8_e4m3
    bfdt = ml_dtypes.bfloat16
    f32 = np.float32
    X = np.ascontiguousarray(np.asarray(X, dtype=f32))
    bq_ = np.asarray(bq, f32)
    bk_ = np.asarray(bk, f32)
    bv_ = np.ascontiguousarray(np.asarray(bv, f32))
    bo_ = np.ascontiguousarray(np.asarray(bo, f32))
    gamma_ = np.ascontiguousarray(np.asarray(gamma, f32))
    beta_ = np.ascontiguousarray(np.asarray(beta, f32))
    flags = (
        bool(np.any(bq_)), bool(np.any(bk_)), bool(np.any(bv_)), bool(np.any(bo_)),
        bool(np.any(gamma_ != 1.0)), bool(np.any(beta_)),
    )

    wqt = np.ascontiguousarray(np.asarray(Wq, f32).T.astype(f8))
    wkt = np.ascontiguousarray(np.asarray(Wk, f32).T.astype(f8))
    wvt = np.ascontiguousarray(np.asarray(Wv, f32).T.astype(f8))
    wot = np.ascontiguousarray(np.asarray(Wo, f32).T.astype(f8))
    bqt = np.ascontiguousarray(bq_.reshape(8, 128).T)
    bkt = np.ascontiguousarray(bk_.reshape(8, 128).T)
    sel8 = np.zeros((8, 4, 128), f32)
    for mm in range(4):
        for p in range(128):
            sel8[2 * mm + p // 64, mm, p] = 1.0
    sel8 = sel8.astype(bfdt)
    ident = np.eye(128, dtype=f32)

    in_maps = []
    for c in range(NCORES):
        b, half = c // 2, c % 2
        Xb = X[b]
        q_rows = Xb[half * SQ : (half + 1) * SQ]
        o_rows = Xb[(1 - half) * SQ : (2 - half) * SQ]
        # queries-first token order (key order is permutation-invariant)
        xt = np.ascontiguousarray(np.concatenate([q_rows, o_rows], axis=0).T.astype(f8))
        in_maps.append(
            {
                "xt": xt,
                "xq": np.ascontiguousarray(q_rows),
                "wqt": wqt,
                "wkt": wkt,
                "wvt": wvt,
                "wot": wot,
                "bqt": bqt,
                "bkt": bkt,
                "bv": bv_,
                "bo": bo_,
                "gamma": gamma_,
                "beta": beta_,
                "sel8": sel8,
                "ident": ident,
            }
        )

    nc = _get_nc(flags)
    res = run_bass_kernel_spmd(nc, in_maps, core_ids=list(range(NCORES)))
    if res.exec_time_ns is not None:
        print(f"HW exec time: {res.exec_time_ns} ns")

    out = np.empty((B, S, D), np.float32)
    for c in range(NCORES):
        b, half = c // 2, c % 2
        out[b, half * SQ : (half + 1) * SQ] = res.results[c]["out"]
    return out
